# revision 33
# baseline (speedup 1.0000x reference)
"""Multi-head attention (QKV proj + per-head bias + softmax + out proj) on 8 TRN2 NeuronCores.

Sharding: data-parallel over batch B=4 x tensor-parallel over head-groups
(12 heads -> 2 groups of 6). core = b*2 + hg. Each core computes its 6 heads'
full attention for one batch element plus the partial output projection over
its heads' rows of W_proj; the two partials per batch are summed on the host
(the deferred all-reduce), where b_proj is also added.

Device-side layout notes:
- Everything runs transposed (feature dim on partitions): x^T, q^T, k^T feed
  the PE directly; softmax runs on S^T tiles [m(keys) x n(queries)] so exp is
  elementwise and the denominator comes free from an extra ones-column in the
  AV matmul's stationary operand ([v | 1] -> U rows 0..63 = unnormalized out,
  row 64 = sum of exp). Normalization multiplies by 1/denom broadcast across
  partitions via gpsimd.partition_broadcast.
- All DRAM inputs stream as bf16 (halves HBM traffic); on-device compute
  stays f32r/f32 (bf16 writes from DVE/ACT measure slower, and bf16 exp
  output slows the Scalar engine by ~13%).
- The steady state is ACT(exp)-bound at ~1.1us/step; PE work per step
  (S pair + AV pair + amortized proj chain) must stay under that. Proj
  matmuls are emitted as back-to-back 3-chains every 4th step. S matmuls
  run two steps ahead so AV's semaphore+ldweights latency hides under them.
- The first 12 attention steps (nb=0, c3=0, mc 0-11) are interleaved into
  the prologue: each xt block's q/k/v unlocks 4 more m-chunks, so ~13us of
  exp work hides under the remaining prologue matmuls.
"""

import numpy as np
import ml_dtypes

import concourse.bacc as bacc
import concourse.tile as tile
from concourse.tile import add_dep_helper
import concourse.mybir as mybir
from concourse.bass_utils import run_bass_kernel_spmd

B, N, C, H, HD = 4, 2048, 768, 12, 64
HL = 6                 # heads per core
CL = HL * HD           # 384 local qkv width
SCALE = HD ** -0.5
P = 128
NB = 512               # query-block (n) size
NBS = N // NB          # 4
MC = N // P            # 16 key-chunks (m)
KC = C // P            # 6 contraction chunks of C
PAIRS = HL // 2        # 3 head pairs (stacked 64+64 on partitions)
D1 = HD + 1            # v augmented with ones column

f32 = mybir.dt.float32
f32r = mybir.dt.float32r
bf16 = mybir.dt.bfloat16
EXP = mybir.ActivationFunctionType.Exp

_CACHE: dict = {}


def _build():
    nc = bacc.Bacc("TRN2", target_bir_lowering=False, debug=False, num_devices=8)

    xt = nc.dram_tensor("xt", [C, N], bf16, kind="ExternalInput")        # x^T
    wq = nc.dram_tensor("wq", [C, CL], bf16, kind="ExternalInput")
    wk = nc.dram_tensor("wk", [C, CL], bf16, kind="ExternalInput")
    wv = nc.dram_tensor("wv", [C, CL], bf16, kind="ExternalInput")
    qb = nc.dram_tensor("qb", [PAIRS, P, N], bf16, kind="ExternalInput")  # qbias^T + b_q
    kb = nc.dram_tensor("kb", [PAIRS, P, N], bf16, kind="ExternalInput")
    vb = nc.dram_tensor("vb", [N, CL], bf16, kind="ExternalInput")        # vbias + b_v
    wp = nc.dram_tensor("wp", [CL, C], bf16, kind="ExternalInput")       # W_proj local rows
    ot = nc.dram_tensor("ot", [C, N], f32, kind="ExternalOutput")        # partial out^T

    xt_r = xt.ap().rearrange("(co p) n -> p co n", p=P)
    wq_r = wq.ap().rearrange("(co p) j -> p co j", p=P)
    wk_r = wk.ap().rearrange("(co p) j -> p co j", p=P)
    wv_r = wv.ap().rearrange("(co p) j -> p co j", p=P)
    wp_r = wp.ap().rearrange("(c3 p) c -> p c3 c", p=P)
    vb_r = vb.ap().rearrange("(mc p) j -> p mc j", p=P)
    ot_r = ot.ap().rearrange("(cc p) n -> p cc n", p=P)

    with tile.TileContext(nc) as tc:
        with (
            tc.tile_pool(name="persist", bufs=1) as pp,
            tc.tile_pool(name="stream", bufs=2) as sp,
            tc.tile_pool(name="ps", bufs=2, space="PSUM") as ps,
        ):
            # ---- persistent tiles ----
            wq_sb = pp.tile([P, KC, CL], bf16)
            wk_sb = pp.tile([P, KC, CL], bf16)
            wv_sb = pp.tile([P, KC, CL], bf16)
            wp_sb = pp.tile([P, PAIRS, C], bf16)
            qT = pp.tile([P, PAIRS, N], f32r)    # q^T (pair-stacked heads)
            kT = pp.tile([P, PAIRS, N], f32r)    # k^T
            v_aug = pp.tile([P, MC, HL, D1], f32r)  # [v | 1] per m-chunk/head
            ones_f32 = pp.tile([P, 1], f32)
            ones_row = pp.tile([1, P], f32)      # stationary for PE broadcast
            pacc = pp.tile([P, C // P, NB], f32)  # nb=3 proj partial acc

            # DMA priority: wq + first xt block feed the first matmuls; wk/wv
            # right behind so the k chains and v chains never wait.
            nc.sync.dma_start(wq_sb[:], wq_r)
            xt_blks = {}

            def fetch_xt(nb):
                t = sp.tile([P, KC, NB], bf16, tag="xt", bufs=2,
                            name=f"xt_{nb}")
                nc.sync.dma_start(t[:], xt_r[:, :, nb * NB:(nb + 1) * NB])
                xt_blks[nb] = t

            fetch_xt(0)

            # PE warmup: a few dense dummy matmuls flip the HAM clock gate
            # toward 2.4 GHz while the first DMAs land; the first real
            # chains finish the ramp.
            warm_a = pp.tile([P, P], f32r)
            warm_b = pp.tile([P, NB], f32r)
            nc.vector.memset(warm_a.bitcast(f32)[:], 0.0)
            nc.vector.memset(warm_b.bitcast(f32)[:], 0.0)
            wps = ps.tile([P, 2, NB], f32, tag="sps", name="warm_ps")
            for _ in range(5):
                nc.tensor.matmul(wps[:, 0, :], warm_a[:], warm_b[:], start=True, stop=True)

            nc.vector.memset(ones_f32[:], 1.0)
            nc.vector.memset(ones_row[:], 1.0)
            with nc.allow_low_precision(reason="ones column is exact in f32r"):
                nc.vector.tensor_copy(
                    v_aug[:, :, :, HD], ones_f32.to_broadcast([P, MC, HL])
                )

            # ---- attention machinery (emission deferred via emit_steps) ----
            steps = [(nb, c3, mc)
                     for nb in range(NBS)
                     for c3 in range(PAIRS)
                     for mc in range(MC)]
            o_blks = {}
            u_cur = {}
            sps_tiles = {}
            exp_tiles = {}
            s_insts = {}

            def emit_s(i):
                nb, c3, mc = steps[i]
                ns = slice(nb * NB, (nb + 1) * NB)
                ms = slice(mc * P, (mc + 1) * P)
                sps = ps.tile([P, 2, NB], f32, tag="sps", name=f"s_{nb}_{c3}_{mc}")
                sps_tiles[i] = sps
                insts = []
                for hp in range(2):
                    hb = slice(hp * HD, (hp + 1) * HD)
                    bi = nc.tensor.matmul(
                        sps[:, hp, :], kT[hb, c3, ms], qT[hb, c3, ns],
                        start=True, stop=True,
                    )
                    insts.append(bi.ins)
                s_insts[i] = insts

            def emit_exp(i):
                nb, c3, mc = steps[i]
                exps = sp.tile([P, 2, NB], f32r, tag="exps", bufs=6,
                               name=f"e_{nb}_{c3}_{mc}")
                exp_tiles[i] = exps
                nc.scalar.activation(exps[:], sps_tiles.pop(i)[:], EXP, scale=SCALE)

            def emit_av(i):
                nb, c3, mc = steps[i]
                if mc == 0:
                    u_cur[0] = ps.tile([D1, NB], f32, tag="ups", name=f"u_{nb}_{c3}_0")
                    u_cur[1] = ps.tile([D1, NB], f32, tag="ups", name=f"u_{nb}_{c3}_1")
                exps = exp_tiles.pop(i)
                for hp in range(2):
                    bi = nc.tensor.matmul(
                        u_cur[hp][:], v_aug[:, mc, c3 * 2 + hp, :],
                        exps[:, hp, :],
                        start=(mc == 0), stop=(mc == MC - 1),
                    )
                    # Pin PE order: the (independent) S matmuls of step i+2
                    # must precede AV(i) in the PE FIFO. Both gate on exp(i)
                    # (AV on the data, S(i+2) on its sps slot), so S(i+2)'s
                    # stream covers AV's ldweights + semaphore latency.
                    if hp == 0 and i + 2 in s_insts:
                        add_dep_helper(bi.ins, s_insts[i + 2][-1], sync=False,
                                       reason="keep S(i+2) ahead of AV(i)")

            def emit_normalize_final(nb, c3):
                # Tail-latency variant for the very last (nb, c3): the
                # multiply reads U straight from PSUM (fusing the staging
                # copy), so the DVE chain is den,den,rec,rec,mul,mul with
                # the broadcasts on gpsimd in between.
                o_blk = o_blks[nb]
                dens, recs, bcs = [], [], []
                for hp in range(2):
                    den = sp.tile([1, NB], f32, tag="den", bufs=4,
                                  name=f"dF_{hp}")
                    nc.vector.tensor_copy(den[:], u_cur[hp][HD:D1, :])
                    dens.append(den)
                for hp in range(2):
                    rec = sp.tile([1, NB], f32, tag="rec", bufs=4,
                                  name=f"rF_{hp}")
                    nc.vector.reciprocal_approx_fast(rec[:], dens[hp][:])
                    bc = sp.tile([P, NB], f32, tag="bc", bufs=3,
                                 name=f"bcF_{hp}")
                    nc.gpsimd.partition_broadcast(bc[:], rec[:])
                    bcs.append(bc)
                for hp in range(2):
                    hb = slice(hp * HD, (hp + 1) * HD)
                    with nc.allow_low_precision(reason="bf16 proj operand"):
                        nc.vector.tensor_mul(
                            o_blk[hb, c3, :], u_cur[hp][0:HD, :], bcs[hp][hb, :]
                        )

            def emit_normalize(nb, c3):
                if nb == NBS - 1 and c3 == PAIRS - 1:
                    emit_normalize_final(nb, c3)
                    return
                # Drain BOTH U psum banks (den + o_blk copies on DVE, with
                # the reciprocal reading the SBUF-staged den), then run the
                # latency-bound broadcast/mul chain off the critical path.
                # partition_broadcast can only write at base 0, so broadcast
                # to all 128 rows and read the 64-row window matching each
                # head's base (SB-SB ops need equal bases).
                o_blk = o_blks[nb]
                recs, bcs = [], []
                for hp in range(2):
                    u = u_cur[hp]
                    hb = slice(hp * HD, (hp + 1) * HD)
                    # den must be staged to SBUF partition 0 first: the
                    # custom-DVE reciprocal cannot read PSUM at partition
                    # base 64 on hardware (CoreSim accepts it; gpsimd cannot
                    # read PSUM at all).
                    den = sp.tile([1, NB], f32, tag="den", bufs=4,
                                  name=f"d_{nb}_{c3}_{hp}")
                    nc.vector.tensor_copy(den[:], u[HD:D1, :])
                    with nc.allow_low_precision(reason="bf16 proj operand"):
                        nc.vector.tensor_copy(o_blk[hb, c3, :], u[0:HD, :])
                    rec = sp.tile([1, NB], f32, tag="rec", bufs=4,
                                  name=f"r_{nb}_{c3}_{hp}")
                    nc.vector.reciprocal_approx_fast(rec[:], den[:])
                    recs.append(rec)
                for hp in range(2):
                    bc = sp.tile([P, NB], f32, tag="bc", bufs=3,
                                 name=f"bc_{nb}_{c3}_{hp}")
                    nc.gpsimd.partition_broadcast(bc[:], recs[hp][:])
                    bcs.append(bc)
                for hp in range(2):
                    hb = slice(hp * HD, (hp + 1) * HD)
                    with nc.allow_low_precision(reason="bf16 proj operand"):
                        nc.vector.tensor_mul(
                            o_blk[hb, c3, :], o_blk[hb, c3, :], bcs[hp][hb, :]
                        )

            # proj(nb) is cut into 12 pieces (6 matmul-chains + 6 drains),
            # one interleaved into every 4th step of nb+1's stream: a chain's
            # 3 matmuls are emitted back-to-back so they pipeline at-rate in
            # the PE, and the amortized per-step PE cost stays under the exp
            # cadence.
            proj_state = {}

            def proj_pieces(nb):
                ns = slice(nb * NB, (nb + 1) * NB)
                o_blk = o_blks.pop(nb)
                for cc in range(C // P):
                    cs = slice(cc * P, (cc + 1) * P)
                    def chain_piece(cc=cc, cs=cs):
                        mm = ps.tile([P, NB], f32, tag="mps", name=f"p_{nb}_{cc}")
                        proj_state["mm"] = mm
                        for c3 in range(PAIRS):
                            nc.tensor.matmul(
                                mm[:], wp_sb[:, c3, cs], o_blk[:, c3, :],
                                start=(c3 == 0), stop=(c3 == PAIRS - 1),
                            )
                    yield chain_piece
                    def out_piece(cc=cc):
                        mm = proj_state.pop("mm")
                        ot_t = sp.tile([P, NB], f32, tag="ot", bufs=3,
                                       name=f"ot_{nb}_{cc}")
                        nc.vector.tensor_copy(ot_t[:], mm[:])
                        nc.sync.dma_start(ot_r[:, cc, ns], ot_t[:])
                    yield out_piece

            # nb=3 has no following steps to host its proj chains, so its
            # c3=0/c3=1 projection partials run during its own c3=1/c3=2
            # steps, accumulated in SBUF; only the 6 c3=2 matmuls + adds
            # remain after the final normalize.
            def tail_phase_pieces(phase):
                o_blk = o_blks[NBS - 1]
                for cc in range(C // P):
                    cs = slice(cc * P, (cc + 1) * P)
                    def piece(cc=cc, cs=cs, phase=phase):
                        mm = ps.tile([P, NB], f32, tag="mps",
                                     name=f"t_{phase}_{cc}")
                        nc.tensor.matmul(
                            mm[:], wp_sb[:, phase, cs],
                            o_blk[:, phase, :], start=True, stop=True,
                        )
                        if phase == 0:
                            nc.vector.tensor_copy(pacc[:, cc, :], mm[:])
                        else:
                            nc.vector.tensor_add(
                                pacc[:, cc, :], pacc[:, cc, :], mm[:]
                            )
                    yield piece

            state = {"si": 0, "next_s": 0, "pending_proj": None,
                     "tail_parts": None, "step_in_nb": 0}

            def emit_steps(upto):
                for i in range(state["si"], upto):
                    nb, c3, mc = steps[i]
                    if mc == 0 and c3 == 0:
                        o_blks[nb] = sp.tile([P, PAIRS, NB], bf16, tag="oblk",
                                             bufs=2, name=f"o_{nb}")
                        if nb > 0:
                            if state["pending_proj"] is not None:
                                for piece in state["pending_proj"]:
                                    piece()
                            state["pending_proj"] = proj_pieces(nb - 1)
                        state["step_in_nb"] = 0
                    while state["next_s"] <= min(i + 2, upto - 1):
                        emit_s(state["next_s"])
                        state["next_s"] += 1
                    emit_exp(i)
                    emit_av(i)
                    # Normalize first on boundary steps so the U-bank drain
                    # ops aren't queued behind a piece's DVE copy.
                    if mc == MC - 1:
                        emit_normalize(nb, c3)
                    # One proj piece every 4th step (after AV so a piece
                    # stalled on its psum slot delays later work, not AV(i)).
                    if (state["pending_proj"] is not None
                            and state["step_in_nb"] >= 2
                            and state["step_in_nb"] % 4 == 2):
                        piece = next(state["pending_proj"], None)
                        if piece is None:
                            state["pending_proj"] = None
                        else:
                            piece()
                    if nb == NBS - 1 and c3 >= 1 and mc == 3:
                        state["tail_parts"] = tail_phase_pieces(c3 - 1)
                    if (state["tail_parts"] is not None
                            and state["step_in_nb"] % 2 == 1):
                        piece = next(state["tail_parts"], None)
                        if piece is None:
                            state["tail_parts"] = None
                        else:
                            piece()
                    state["step_in_nb"] += 1
                state["si"] = upto

            # ---- prologue: stream x^T in n-blocks; produce q^T, k^T, v.
            # After each block, the newly-unlocked (nb=0, c3=0) attention
            # steps are emitted so their exps run under later blocks' PE
            # work. ----
            for nb in range(NBS):
                ns = slice(nb * NB, (nb + 1) * NB)
                xt_blk = xt_blks.pop(nb)
                if nb > 0 and nb + 1 < NBS:
                    fetch_xt(nb + 1)

                # nb=0 runs all q chains before the k chains so the k matmuls
                # never wait on the wk DMA (which is queued after wq + xt0);
                # the xt1 prefetch is also deferred past the q chains so it
                # doesn't steal DMA bandwidth from wk.
                if nb == 0:
                    qk_order = [(wq_sb, qb, qT, c3) for c3 in range(PAIRS)] + \
                               [(None, None, None, -1)] + \
                               [(wk_sb, kb, kT, c3) for c3 in range(PAIRS)]
                else:
                    qk_order = [(w, bd, d, c3) for c3 in range(PAIRS)
                                for (w, bd, d) in ((wq_sb, qb, qT), (wk_sb, kb, kT))]
                for (w_sb, bias_dram, dst, c3) in qk_order:
                    if w_sb is None:
                        # wk/wv DMAs deferred here so wq + xt0 own the
                        # queues during the warmup window.
                        nc.sync.dma_start(wk_sb[:], wk_r)
                        nc.sync.dma_start(wv_sb[:], wv_r)
                        fetch_xt(1)
                        continue
                    js = slice(c3 * P, (c3 + 1) * P)
                    mm = ps.tile([P, NB], f32, tag="mps", name=f"qk_{nb}_{c3}")
                    for co in range(KC):
                        nc.tensor.matmul(
                            mm[:], w_sb[:, co, js], xt_blk[:, co, :],
                            start=(co == 0), stop=(co == KC - 1),
                        )
                    bias_t = sp.tile([P, NB], bf16, tag="bias", bufs=4,
                                     name=f"b_{nb}_{c3}")
                    nc.sync.dma_start(bias_t[:], bias_dram.ap()[c3, :, ns])
                    with nc.allow_low_precision(reason="f32r matmul operand"):
                        nc.vector.tensor_add(dst[:, c3, ns], mm[:], bias_t[:])

                for ch in range(NB // P):
                    mchunk = nb * (NB // P) + ch
                    cs = slice(ch * P, (ch + 1) * P)
                    mm = ps.tile([P, NB], f32, tag="mps", name=f"v_{nb}_{ch}")
                    for co in range(KC):
                        nc.tensor.matmul(
                            mm[:, :CL], xt_blk[:, co, cs], wv_sb[:, co, :],
                            start=(co == 0), stop=(co == KC - 1),
                        )
                    vb_t = sp.tile([P, CL], bf16, tag="vb", bufs=4,
                                   name=f"vb_{mchunk}")
                    nc.sync.dma_start(vb_t[:], vb_r[:, mchunk, :])
                    with nc.allow_low_precision(reason="f32r matmul operand"):
                        nc.vector.tensor_add(
                            v_aug[:, mchunk, :, 0:HD], mm[:, :CL], vb_t[:]
                        )

                # steps (0, 0, mc) unlock 4 m-chunks per completed xt block
                emit_steps(4 * (nb + 1) if nb < NBS - 1 else 12)
            nc.sync.dma_start(wp_sb[:], wp_r)

            # ---- remaining attention + projection ----
            emit_steps(len(steps))

            # drain remaining pieces, then close the last n-block's
            # projection: 6 single matmuls on the final c3 plus the SBUF
            # accumulator.
            if state["pending_proj"] is not None:
                for piece in state["pending_proj"]:
                    piece()
            if state["tail_parts"] is not None:
                for piece in state["tail_parts"]:
                    piece()
            # The final 6 matmuls drain as 3 pairs through the (now free)
            # sps psum tiles, each closed by one wide add + one wide DMA.
            o_blk3 = o_blks.pop(NBS - 1)
            ns3 = slice((NBS - 1) * NB, NBS * NB)
            for pc in range(C // P // 2):
                mm = ps.tile([P, 2, NB], f32, tag="sps", name=f"tf_{pc}")
                for j in range(2):
                    cc = 2 * pc + j
                    cs = slice(cc * P, (cc + 1) * P)
                    nc.tensor.matmul(mm[:, j, :], wp_sb[:, 2, cs],
                                     o_blk3[:, 2, :], start=True, stop=True)
                ot_t = sp.tile([P, 2, NB], f32, tag="otw", bufs=2,
                               name=f"ot_3_{pc}")
                nc.vector.tensor_add(
                    ot_t[:], pacc[:, 2 * pc:2 * pc + 2, :], mm[:]
                )
                nc.sync.dma_start(ot_r[:, 2 * pc:2 * pc + 2, ns3], ot_t[:])

    nc.compile()
    return nc


def _get_nc():
    if "nc" not in _CACHE:
        _CACHE["nc"] = _build()
    return _CACHE["nc"]


def _prep_in_maps(x, qbias, kbias, vbias, W_qkv, b_qkv, W_proj):
    x = np.asarray(x, dtype=np.float32)
    qbias = np.asarray(qbias, dtype=np.float32)
    kbias = np.asarray(kbias, dtype=np.float32)
    vbias = np.asarray(vbias, dtype=np.float32)
    W_qkv = np.asarray(W_qkv, dtype=np.float32)
    b_qkv = np.asarray(b_qkv, dtype=np.float32)
    W_proj = np.asarray(W_proj, dtype=np.float32)
    b16 = ml_dtypes.bfloat16

    xts = [np.ascontiguousarray(x[b].T).astype(b16) for b in range(B)]
    in_maps = []
    for core in range(8):
        b, hg = core // 2, core % 2
        heads = slice(hg * HL, (hg + 1) * HL)
        qcols = slice(hg * CL, (hg + 1) * CL)
        kcols = slice(C + hg * CL, C + (hg + 1) * CL)
        vcols = slice(2 * C + hg * CL, 2 * C + (hg + 1) * CL)

        # per-head bias + projection bias, transposed to [pair, 128, N]
        qb_ = qbias[b, heads] + b_qkv[qcols].reshape(HL, 1, HD)   # [6, N, 64]
        kb_ = kbias[b, heads] + b_qkv[kcols].reshape(HL, 1, HD)
        qb_t = np.ascontiguousarray(qb_.transpose(0, 2, 1)).reshape(PAIRS, P, N)
        kb_t = np.ascontiguousarray(kb_.transpose(0, 2, 1)).reshape(PAIRS, P, N)
        # v bias in natural [N, 384] (heads side by side, matching Wv columns)
        vb_ = vbias[b, heads] + b_qkv[vcols].reshape(HL, 1, HD)   # [6, N, 64]
        vb_n = np.ascontiguousarray(vb_.transpose(1, 0, 2)).reshape(N, CL)

        in_maps.append({
            "xt": xts[b],
            "wq": np.ascontiguousarray(W_qkv[:, qcols]).astype(b16),
            "wk": np.ascontiguousarray(W_qkv[:, kcols]).astype(b16),
            "wv": np.ascontiguousarray(W_qkv[:, vcols]).astype(b16),
            "qb": qb_t.astype(b16),
            "kb": kb_t.astype(b16),
            "vb": vb_n.astype(b16),
            "wp": np.ascontiguousarray(W_proj[hg * CL:(hg + 1) * CL, :]).astype(b16),
        })
    return in_maps


def kernel(x, qbias, kbias, vbias, W_qkv, b_qkv, W_proj, b_proj, **run_kwargs):
    nc = _get_nc()
    in_maps = _prep_in_maps(x, qbias, kbias, vbias, W_qkv, b_qkv, W_proj)
    res = run_bass_kernel_spmd(nc, in_maps, core_ids=list(range(8)), **run_kwargs)
    _CACHE["last_results"] = res

    b_proj = np.asarray(b_proj, dtype=np.float32)
    out = np.empty((B, N, C), dtype=np.float32)
    for b in range(B):
        part = res.results[2 * b]["ot"] + res.results[2 * b + 1]["ot"]  # [C, N]
        out[b] = part.T + b_proj
    return out


# revision 34
# speedup vs baseline: 1.1800x; 1.1800x over previous
"""Multi-head attention (QKV proj + per-head bias + softmax + out proj) on 8 TRN2 NeuronCores.

Sharding: data-parallel over batch B=4 x tensor-parallel over head-groups
(12 heads -> 2 groups of 6). core = b*2 + hg. Each core computes its 6 heads'
full attention for one batch element plus the partial output projection over
its heads' rows of W_proj; the two partials per batch are summed on the host
(the deferred all-reduce), where b_proj is also added.

Device-side layout notes:
- Everything runs transposed (feature dim on partitions): x^T, q^T, k^T feed
  the PE directly; softmax runs on S^T tiles [m(keys) x n(queries)] so exp is
  elementwise and the denominator comes free from an extra ones-column in the
  AV matmul's stationary operand ([v | 1] -> U rows 0..63 = unnormalized out,
  row 64 = sum of exp). Normalization multiplies by 1/denom broadcast across
  partitions via gpsimd.partition_broadcast.
- All DRAM inputs stream as bf16 (halves HBM traffic); on-device compute
  stays f32r/f32 (bf16 writes from DVE/ACT measure slower, and bf16 exp
  output slows the Scalar engine by ~13%).
- The steady state is ACT(exp)-bound at ~1.1us/step; PE work per step
  (S pair + AV pair + amortized proj chain) must stay under that. Proj
  matmuls are emitted as back-to-back 3-chains every 4th step. S matmuls
  run two steps ahead so AV's semaphore+ldweights latency hides under them.
- The first 12 attention steps (nb=0, c3=0, mc 0-11) are interleaved into
  the prologue: each xt block's q/k/v unlocks 4 more m-chunks, so ~13us of
  exp work hides under the remaining prologue matmuls.
"""

import numpy as np
import ml_dtypes

import concourse.bacc as bacc
import concourse.tile as tile
from concourse.tile import add_dep_helper
import concourse.mybir as mybir
from concourse.bass_utils import run_bass_kernel_spmd

B, N, C, H, HD = 4, 2048, 768, 12, 64
HL = 6                 # heads per core
CL = HL * HD           # 384 local qkv width
SCALE = HD ** -0.5
P = 128
NB = 512               # query-block (n) size
NBS = N // NB          # 4
MC = N // P            # 16 key-chunks (m)
KC = C // P            # 6 contraction chunks of C
PAIRS = HL // 2        # 3 head pairs (stacked 64+64 on partitions)
D1 = HD + 1            # v augmented with ones column

f32 = mybir.dt.float32
f32r = mybir.dt.float32r
bf16 = mybir.dt.bfloat16
EXP = mybir.ActivationFunctionType.Exp

_CACHE: dict = {}


def _build():
    nc = bacc.Bacc("TRN2", target_bir_lowering=False, debug=False, num_devices=8)

    xt = nc.dram_tensor("xt", [C, N], bf16, kind="ExternalInput")        # x^T
    wq = nc.dram_tensor("wq", [C, CL], bf16, kind="ExternalInput")
    wk = nc.dram_tensor("wk", [C, CL], bf16, kind="ExternalInput")
    wv = nc.dram_tensor("wv", [C, CL], bf16, kind="ExternalInput")
    qb = nc.dram_tensor("qb", [PAIRS, P, N], bf16, kind="ExternalInput")  # qbias^T + b_q
    kb = nc.dram_tensor("kb", [PAIRS, P, N], bf16, kind="ExternalInput")
    vb = nc.dram_tensor("vb", [N, CL], bf16, kind="ExternalInput")        # vbias + b_v
    wp = nc.dram_tensor("wp", [CL, C], bf16, kind="ExternalInput")       # W_proj local rows
    ot = nc.dram_tensor("ot", [C, N], f32, kind="ExternalOutput")        # partial out^T

    xt_r = xt.ap().rearrange("(co p) n -> p co n", p=P)
    wq_r = wq.ap().rearrange("(co p) j -> p co j", p=P)
    wk_r = wk.ap().rearrange("(co p) j -> p co j", p=P)
    wv_r = wv.ap().rearrange("(co p) j -> p co j", p=P)
    wp_r = wp.ap().rearrange("(c3 p) c -> p c3 c", p=P)
    vb_r = vb.ap().rearrange("(mc p) j -> p mc j", p=P)
    ot_r = ot.ap().rearrange("(cc p) n -> p cc n", p=P)

    with tile.TileContext(nc) as tc:
        with (
            tc.tile_pool(name="persist", bufs=1) as pp,
            tc.tile_pool(name="stream", bufs=2) as sp,
            tc.tile_pool(name="ps", bufs=2, space="PSUM") as ps,
        ):
            # ---- persistent tiles ----
            wq_sb = pp.tile([P, KC, CL], bf16)
            wk_sb = pp.tile([P, KC, CL], bf16)
            wv_sb = pp.tile([P, KC, CL], bf16)
            wp_sb = pp.tile([P, PAIRS, C], bf16)
            qT = pp.tile([P, PAIRS, N], f32r)    # q^T (pair-stacked heads)
            kT = pp.tile([P, PAIRS, N], f32r)    # k^T
            v_aug = pp.tile([P, MC, HL, D1], f32r)  # [v | 1] per m-chunk/head
            ones_f32 = pp.tile([P, 1], f32)
            ones_row = pp.tile([1, P], f32)      # stationary for PE broadcast
            pacc = pp.tile([P, C // P, NB], f32)  # nb=3 proj partial acc

            # DMA priority: wq + first xt block feed the first matmuls; wk/wv
            # right behind so the k chains and v chains never wait.
            nc.sync.dma_start(wq_sb[:], wq_r)
            xt_blks = {}

            def fetch_xt(nb):
                t = sp.tile([P, KC, NB], bf16, tag="xt", bufs=2,
                            name=f"xt_{nb}")
                nc.sync.dma_start(t[:], xt_r[:, :, nb * NB:(nb + 1) * NB])
                xt_blks[nb] = t

            fetch_xt(0)

            # PE warmup: ~4us of dense dummy matmuls flips the HAM clock
            # gate to 8/8 (2.4 GHz) while the first DMAs land. (Fewer than
            # ~10 measurably re-throttles the clock: 5 warmups cost +60us.)
            warm_a = pp.tile([P, P], f32r)
            warm_b = pp.tile([P, NB], f32r)
            nc.vector.memset(warm_a.bitcast(f32)[:], 0.0)
            nc.vector.memset(warm_b.bitcast(f32)[:], 0.0)
            wps = ps.tile([P, 2, NB], f32, tag="sps", name="warm_ps")
            for _ in range(11):
                nc.tensor.matmul(wps[:, 0, :], warm_a[:], warm_b[:], start=True, stop=True)

            nc.vector.memset(ones_f32[:], 1.0)
            nc.vector.memset(ones_row[:], 1.0)
            with nc.allow_low_precision(reason="ones column is exact in f32r"):
                nc.vector.tensor_copy(
                    v_aug[:, :, :, HD], ones_f32.to_broadcast([P, MC, HL])
                )

            # ---- attention machinery (emission deferred via emit_steps) ----
            steps = [(nb, c3, mc)
                     for nb in range(NBS)
                     for c3 in range(PAIRS)
                     for mc in range(MC)]
            o_blks = {}
            u_cur = {}
            sps_tiles = {}
            exp_tiles = {}
            s_insts = {}

            def emit_s(i):
                nb, c3, mc = steps[i]
                ns = slice(nb * NB, (nb + 1) * NB)
                ms = slice(mc * P, (mc + 1) * P)
                sps = ps.tile([P, 2, NB], f32, tag="sps", name=f"s_{nb}_{c3}_{mc}")
                sps_tiles[i] = sps
                insts = []
                for hp in range(2):
                    hb = slice(hp * HD, (hp + 1) * HD)
                    bi = nc.tensor.matmul(
                        sps[:, hp, :], kT[hb, c3, ms], qT[hb, c3, ns],
                        start=True, stop=True,
                    )
                    insts.append(bi.ins)
                s_insts[i] = insts

            def emit_exp(i):
                nb, c3, mc = steps[i]
                exps = sp.tile([P, 2, NB], f32r, tag="exps", bufs=6,
                               name=f"e_{nb}_{c3}_{mc}")
                exp_tiles[i] = exps
                nc.scalar.activation(exps[:], sps_tiles.pop(i)[:], EXP, scale=SCALE)

            def emit_av(i):
                nb, c3, mc = steps[i]
                if mc == 0:
                    u_cur[0] = ps.tile([D1, NB], f32, tag="ups", name=f"u_{nb}_{c3}_0")
                    u_cur[1] = ps.tile([D1, NB], f32, tag="ups", name=f"u_{nb}_{c3}_1")
                exps = exp_tiles.pop(i)
                for hp in range(2):
                    bi = nc.tensor.matmul(
                        u_cur[hp][:], v_aug[:, mc, c3 * 2 + hp, :],
                        exps[:, hp, :],
                        start=(mc == 0), stop=(mc == MC - 1),
                    )
                    # Pin PE order: the (independent) S matmuls of step i+2
                    # must precede AV(i) in the PE FIFO. Both gate on exp(i)
                    # (AV on the data, S(i+2) on its sps slot), so S(i+2)'s
                    # stream covers AV's ldweights + semaphore latency.
                    if hp == 0 and i + 2 in s_insts:
                        add_dep_helper(bi.ins, s_insts[i + 2][-1], sync=False,
                                       reason="keep S(i+2) ahead of AV(i)")

            def emit_normalize_final(nb, c3):
                # Tail-latency variant for the very last (nb, c3): the
                # multiply reads U straight from PSUM (fusing the staging
                # copy), so the DVE chain is den,den,rec,rec,mul,mul with
                # the broadcasts on gpsimd in between.
                o_blk = o_blks[nb]
                dens, recs, bcs = [], [], []
                for hp in range(2):
                    den = sp.tile([1, NB], f32, tag="den", bufs=4,
                                  name=f"dF_{hp}")
                    nc.vector.tensor_copy(den[:], u_cur[hp][HD:D1, :])
                    dens.append(den)
                for hp in range(2):
                    rec = sp.tile([1, NB], f32, tag="rec", bufs=4,
                                  name=f"rF_{hp}")
                    nc.vector.reciprocal_approx_fast(rec[:], dens[hp][:])
                    bc = sp.tile([P, NB], f32, tag="bc", bufs=3,
                                 name=f"bcF_{hp}")
                    nc.gpsimd.partition_broadcast(bc[:], rec[:])
                    bcs.append(bc)
                for hp in range(2):
                    hb = slice(hp * HD, (hp + 1) * HD)
                    with nc.allow_low_precision(reason="bf16 proj operand"):
                        nc.vector.tensor_mul(
                            o_blk[hb, c3, :], u_cur[hp][0:HD, :], bcs[hp][hb, :]
                        )

            def emit_normalize(nb, c3):
                if nb == NBS - 1 and c3 == PAIRS - 1:
                    emit_normalize_final(nb, c3)
                    return
                # Drain BOTH U psum banks (den + o_blk copies on DVE, with
                # the reciprocal reading the SBUF-staged den), then run the
                # latency-bound broadcast/mul chain off the critical path.
                # partition_broadcast can only write at base 0, so broadcast
                # to all 128 rows and read the 64-row window matching each
                # head's base (SB-SB ops need equal bases).
                o_blk = o_blks[nb]
                recs, bcs = [], []
                for hp in range(2):
                    u = u_cur[hp]
                    hb = slice(hp * HD, (hp + 1) * HD)
                    # den must be staged to SBUF partition 0 first: the
                    # custom-DVE reciprocal cannot read PSUM at partition
                    # base 64 on hardware (CoreSim accepts it; gpsimd cannot
                    # read PSUM at all).
                    den = sp.tile([1, NB], f32, tag="den", bufs=4,
                                  name=f"d_{nb}_{c3}_{hp}")
                    nc.vector.tensor_copy(den[:], u[HD:D1, :])
                    with nc.allow_low_precision(reason="bf16 proj operand"):
                        nc.vector.tensor_copy(o_blk[hb, c3, :], u[0:HD, :])
                    rec = sp.tile([1, NB], f32, tag="rec", bufs=4,
                                  name=f"r_{nb}_{c3}_{hp}")
                    nc.vector.reciprocal_approx_fast(rec[:], den[:])
                    recs.append(rec)
                for hp in range(2):
                    bc = sp.tile([P, NB], f32, tag="bc", bufs=3,
                                 name=f"bc_{nb}_{c3}_{hp}")
                    nc.gpsimd.partition_broadcast(bc[:], recs[hp][:])
                    bcs.append(bc)
                for hp in range(2):
                    hb = slice(hp * HD, (hp + 1) * HD)
                    with nc.allow_low_precision(reason="bf16 proj operand"):
                        nc.vector.tensor_mul(
                            o_blk[hb, c3, :], o_blk[hb, c3, :], bcs[hp][hb, :]
                        )

            # proj(nb) is cut into 12 pieces (6 matmul-chains + 6 drains),
            # one interleaved into every 4th step of nb+1's stream: a chain's
            # 3 matmuls are emitted back-to-back so they pipeline at-rate in
            # the PE, and the amortized per-step PE cost stays under the exp
            # cadence.
            proj_state = {}

            def proj_pieces(nb):
                ns = slice(nb * NB, (nb + 1) * NB)
                o_blk = o_blks.pop(nb)
                for cc in range(C // P):
                    cs = slice(cc * P, (cc + 1) * P)
                    def chain_piece(cc=cc, cs=cs):
                        mm = ps.tile([P, NB], f32, tag="mps", name=f"p_{nb}_{cc}")
                        proj_state["mm"] = mm
                        for c3 in range(PAIRS):
                            nc.tensor.matmul(
                                mm[:], wp_sb[:, c3, cs], o_blk[:, c3, :],
                                start=(c3 == 0), stop=(c3 == PAIRS - 1),
                            )
                    yield chain_piece
                    def out_piece(cc=cc):
                        mm = proj_state.pop("mm")
                        ot_t = sp.tile([P, NB], f32, tag="ot", bufs=3,
                                       name=f"ot_{nb}_{cc}")
                        nc.vector.tensor_copy(ot_t[:], mm[:])
                        nc.sync.dma_start(ot_r[:, cc, ns], ot_t[:])
                    yield out_piece

            # nb=3 has no following steps to host its proj chains, so its
            # c3=0/c3=1 projection partials run during its own c3=1/c3=2
            # steps, accumulated in SBUF; only the 6 c3=2 matmuls + adds
            # remain after the final normalize.
            def tail_phase_pieces(phase):
                o_blk = o_blks[NBS - 1]
                for cc in range(C // P):
                    cs = slice(cc * P, (cc + 1) * P)
                    def piece(cc=cc, cs=cs, phase=phase):
                        mm = ps.tile([P, NB], f32, tag="mps",
                                     name=f"t_{phase}_{cc}")
                        nc.tensor.matmul(
                            mm[:], wp_sb[:, phase, cs],
                            o_blk[:, phase, :], start=True, stop=True,
                        )
                        if phase == 0:
                            nc.vector.tensor_copy(pacc[:, cc, :], mm[:])
                        else:
                            nc.vector.tensor_add(
                                pacc[:, cc, :], pacc[:, cc, :], mm[:]
                            )
                    yield piece

            state = {"si": 0, "next_s": 0, "pending_proj": None,
                     "tail_parts": None, "step_in_nb": 0}

            def emit_steps(upto):
                for i in range(state["si"], upto):
                    nb, c3, mc = steps[i]
                    if mc == 0 and c3 == 0:
                        o_blks[nb] = sp.tile([P, PAIRS, NB], bf16, tag="oblk",
                                             bufs=2, name=f"o_{nb}")
                        if nb > 0:
                            if state["pending_proj"] is not None:
                                for piece in state["pending_proj"]:
                                    piece()
                            state["pending_proj"] = proj_pieces(nb - 1)
                        state["step_in_nb"] = 0
                    while state["next_s"] <= min(i + 2, upto - 1):
                        emit_s(state["next_s"])
                        state["next_s"] += 1
                    emit_exp(i)
                    emit_av(i)
                    # Normalize first on boundary steps so the U-bank drain
                    # ops aren't queued behind a piece's DVE copy.
                    if mc == MC - 1:
                        emit_normalize(nb, c3)
                    # One proj piece every 4th step (after AV so a piece
                    # stalled on its psum slot delays later work, not AV(i)).
                    if (state["pending_proj"] is not None
                            and state["step_in_nb"] >= 2
                            and state["step_in_nb"] % 4 == 2):
                        piece = next(state["pending_proj"], None)
                        if piece is None:
                            state["pending_proj"] = None
                        else:
                            piece()
                    if nb == NBS - 1 and c3 >= 1 and mc == 3:
                        state["tail_parts"] = tail_phase_pieces(c3 - 1)
                    if (state["tail_parts"] is not None
                            and state["step_in_nb"] % 2 == 1):
                        piece = next(state["tail_parts"], None)
                        if piece is None:
                            state["tail_parts"] = None
                        else:
                            piece()
                    state["step_in_nb"] += 1
                state["si"] = upto

            # ---- prologue: stream x^T in n-blocks; produce q^T, k^T, v.
            # After each block, the newly-unlocked (nb=0, c3=0) attention
            # steps are emitted so their exps run under later blocks' PE
            # work. ----
            for nb in range(NBS):
                ns = slice(nb * NB, (nb + 1) * NB)
                xt_blk = xt_blks.pop(nb)
                if nb > 0 and nb + 1 < NBS:
                    fetch_xt(nb + 1)

                # nb=0 runs all q chains before the k chains so the k matmuls
                # never wait on the wk DMA (which is queued after wq + xt0);
                # the xt1 prefetch is also deferred past the q chains so it
                # doesn't steal DMA bandwidth from wk.
                if nb == 0:
                    qk_order = [(wq_sb, qb, qT, c3) for c3 in range(PAIRS)] + \
                               [(None, None, None, -1)] + \
                               [(wk_sb, kb, kT, c3) for c3 in range(PAIRS)]
                else:
                    qk_order = [(w, bd, d, c3) for c3 in range(PAIRS)
                                for (w, bd, d) in ((wq_sb, qb, qT), (wk_sb, kb, kT))]
                for (w_sb, bias_dram, dst, c3) in qk_order:
                    if w_sb is None:
                        # wk/wv DMAs deferred here so wq + xt0 own the
                        # queues during the warmup window.
                        nc.sync.dma_start(wk_sb[:], wk_r)
                        nc.sync.dma_start(wv_sb[:], wv_r)
                        fetch_xt(1)
                        continue
                    js = slice(c3 * P, (c3 + 1) * P)
                    mm = ps.tile([P, NB], f32, tag="mps", name=f"qk_{nb}_{c3}")
                    for co in range(KC):
                        nc.tensor.matmul(
                            mm[:], w_sb[:, co, js], xt_blk[:, co, :],
                            start=(co == 0), stop=(co == KC - 1),
                        )
                    bias_t = sp.tile([P, NB], bf16, tag="bias", bufs=4,
                                     name=f"b_{nb}_{c3}")
                    nc.sync.dma_start(bias_t[:], bias_dram.ap()[c3, :, ns])
                    with nc.allow_low_precision(reason="f32r matmul operand"):
                        nc.vector.tensor_add(dst[:, c3, ns], mm[:], bias_t[:])

                for ch in range(NB // P):
                    mchunk = nb * (NB // P) + ch
                    cs = slice(ch * P, (ch + 1) * P)
                    mm = ps.tile([P, NB], f32, tag="mps", name=f"v_{nb}_{ch}")
                    for co in range(KC):
                        nc.tensor.matmul(
                            mm[:, :CL], xt_blk[:, co, cs], wv_sb[:, co, :],
                            start=(co == 0), stop=(co == KC - 1),
                        )
                    vb_t = sp.tile([P, CL], bf16, tag="vb", bufs=4,
                                   name=f"vb_{mchunk}")
                    nc.sync.dma_start(vb_t[:], vb_r[:, mchunk, :])
                    with nc.allow_low_precision(reason="f32r matmul operand"):
                        nc.vector.tensor_add(
                            v_aug[:, mchunk, :, 0:HD], mm[:, :CL], vb_t[:]
                        )

                # steps (0, 0, mc) unlock 4 m-chunks per completed xt block
                emit_steps(4 * (nb + 1) if nb < NBS - 1 else 12)
            nc.sync.dma_start(wp_sb[:], wp_r)

            # ---- remaining attention + projection ----
            emit_steps(len(steps))

            # drain remaining pieces, then close the last n-block's
            # projection: 6 single matmuls on the final c3 plus the SBUF
            # accumulator.
            if state["pending_proj"] is not None:
                for piece in state["pending_proj"]:
                    piece()
            if state["tail_parts"] is not None:
                for piece in state["tail_parts"]:
                    piece()
            # The final 6 matmuls drain as 3 pairs through the (now free)
            # sps psum tiles, each closed by one wide add + one wide DMA.
            o_blk3 = o_blks.pop(NBS - 1)
            ns3 = slice((NBS - 1) * NB, NBS * NB)
            for pc in range(C // P // 2):
                mm = ps.tile([P, 2, NB], f32, tag="sps", name=f"tf_{pc}")
                for j in range(2):
                    cc = 2 * pc + j
                    cs = slice(cc * P, (cc + 1) * P)
                    nc.tensor.matmul(mm[:, j, :], wp_sb[:, 2, cs],
                                     o_blk3[:, 2, :], start=True, stop=True)
                ot_t = sp.tile([P, 2, NB], f32, tag="otw", bufs=2,
                               name=f"ot_3_{pc}")
                nc.vector.tensor_add(
                    ot_t[:], pacc[:, 2 * pc:2 * pc + 2, :], mm[:]
                )
                nc.sync.dma_start(ot_r[:, 2 * pc:2 * pc + 2, ns3], ot_t[:])

    nc.compile()
    return nc


def _get_nc():
    if "nc" not in _CACHE:
        _CACHE["nc"] = _build()
    return _CACHE["nc"]


def _prep_in_maps(x, qbias, kbias, vbias, W_qkv, b_qkv, W_proj):
    x = np.asarray(x, dtype=np.float32)
    qbias = np.asarray(qbias, dtype=np.float32)
    kbias = np.asarray(kbias, dtype=np.float32)
    vbias = np.asarray(vbias, dtype=np.float32)
    W_qkv = np.asarray(W_qkv, dtype=np.float32)
    b_qkv = np.asarray(b_qkv, dtype=np.float32)
    W_proj = np.asarray(W_proj, dtype=np.float32)
    b16 = ml_dtypes.bfloat16

    xts = [np.ascontiguousarray(x[b].T).astype(b16) for b in range(B)]
    in_maps = []
    for core in range(8):
        b, hg = core // 2, core % 2
        heads = slice(hg * HL, (hg + 1) * HL)
        qcols = slice(hg * CL, (hg + 1) * CL)
        kcols = slice(C + hg * CL, C + (hg + 1) * CL)
        vcols = slice(2 * C + hg * CL, 2 * C + (hg + 1) * CL)

        # per-head bias + projection bias, transposed to [pair, 128, N]
        qb_ = qbias[b, heads] + b_qkv[qcols].reshape(HL, 1, HD)   # [6, N, 64]
        kb_ = kbias[b, heads] + b_qkv[kcols].reshape(HL, 1, HD)
        qb_t = np.ascontiguousarray(qb_.transpose(0, 2, 1)).reshape(PAIRS, P, N)
        kb_t = np.ascontiguousarray(kb_.transpose(0, 2, 1)).reshape(PAIRS, P, N)
        # v bias in natural [N, 384] (heads side by side, matching Wv columns)
        vb_ = vbias[b, heads] + b_qkv[vcols].reshape(HL, 1, HD)   # [6, N, 64]
        vb_n = np.ascontiguousarray(vb_.transpose(1, 0, 2)).reshape(N, CL)

        in_maps.append({
            "xt": xts[b],
            "wq": np.ascontiguousarray(W_qkv[:, qcols]).astype(b16),
            "wk": np.ascontiguousarray(W_qkv[:, kcols]).astype(b16),
            "wv": np.ascontiguousarray(W_qkv[:, vcols]).astype(b16),
            "qb": qb_t.astype(b16),
            "kb": kb_t.astype(b16),
            "vb": vb_n.astype(b16),
            "wp": np.ascontiguousarray(W_proj[hg * CL:(hg + 1) * CL, :]).astype(b16),
        })
    return in_maps


def kernel(x, qbias, kbias, vbias, W_qkv, b_qkv, W_proj, b_proj, **run_kwargs):
    nc = _get_nc()
    in_maps = _prep_in_maps(x, qbias, kbias, vbias, W_qkv, b_qkv, W_proj)
    res = run_bass_kernel_spmd(nc, in_maps, core_ids=list(range(8)), **run_kwargs)
    _CACHE["last_results"] = res

    b_proj = np.asarray(b_proj, dtype=np.float32)
    out = np.empty((B, N, C), dtype=np.float32)
    for b in range(B):
        part = res.results[2 * b]["ot"] + res.results[2 * b + 1]["ot"]  # [C, N]
        out[b] = part.T + b_proj
    return out


# revision 37
# speedup vs baseline: 1.1804x; 1.0003x over previous
"""Multi-head attention (QKV proj + per-head bias + softmax + out proj) on 8 TRN2 NeuronCores.

Sharding: data-parallel over batch B=4 x tensor-parallel over head-groups
(12 heads -> 2 groups of 6). core = b*2 + hg. Each core computes its 6 heads'
full attention for one batch element plus the partial output projection over
its heads' rows of W_proj; the two partials per batch are summed on the host
(the deferred all-reduce), where b_proj is also added.

Device-side layout notes:
- Everything runs transposed (feature dim on partitions): x^T, q^T, k^T feed
  the PE directly; softmax runs on S^T tiles [m(keys) x n(queries)] so exp is
  elementwise and the denominator comes free from an extra ones-column in the
  AV matmul's stationary operand ([v | 1] -> U rows 0..63 = unnormalized out,
  row 64 = sum of exp). Normalization multiplies by 1/denom broadcast across
  partitions via gpsimd.partition_broadcast.
- All DRAM inputs stream as bf16 (halves HBM traffic); on-device compute
  stays f32r/f32 (bf16 writes from DVE/ACT measure slower, and bf16 exp
  output slows the Scalar engine by ~13%).
- The steady state is ACT(exp)-bound at ~1.1us/step; PE work per step
  (S pair + AV pair + amortized proj chain) must stay under that. Proj
  matmuls are emitted as back-to-back 3-chains every 4th step. S matmuls
  run two steps ahead so AV's semaphore+ldweights latency hides under them.
- The first 12 attention steps (nb=0, c3=0, mc 0-11) are interleaved into
  the prologue: each xt block's q/k/v unlocks 4 more m-chunks, so ~13us of
  exp work hides under the remaining prologue matmuls.
"""

import numpy as np
import ml_dtypes

import concourse.bacc as bacc
import concourse.tile as tile
from concourse.tile import add_dep_helper
import concourse.mybir as mybir
from concourse.bass_utils import run_bass_kernel_spmd

B, N, C, H, HD = 4, 2048, 768, 12, 64
HL = 6                 # heads per core
CL = HL * HD           # 384 local qkv width
SCALE = HD ** -0.5
P = 128
NB = 512               # query-block (n) size
NBS = N // NB          # 4
MC = N // P            # 16 key-chunks (m)
KC = C // P            # 6 contraction chunks of C
PAIRS = HL // 2        # 3 head pairs (stacked 64+64 on partitions)
D1 = HD + 1            # v augmented with ones column

f32 = mybir.dt.float32
f32r = mybir.dt.float32r
bf16 = mybir.dt.bfloat16
EXP = mybir.ActivationFunctionType.Exp

_CACHE: dict = {}


def _build():
    nc = bacc.Bacc("TRN2", target_bir_lowering=False, debug=False, num_devices=8)

    xt = nc.dram_tensor("xt", [C, N], bf16, kind="ExternalInput")        # x^T
    # Weights arrive pre-transposed to the on-chip [partition, chunk, cols]
    # layout so each partition's DMA is one contiguous burst.
    wq = nc.dram_tensor("wq", [P, KC, CL], bf16, kind="ExternalInput")
    wk = nc.dram_tensor("wk", [P, KC, CL], bf16, kind="ExternalInput")
    wv = nc.dram_tensor("wv", [P, KC, CL], bf16, kind="ExternalInput")
    qb = nc.dram_tensor("qb", [PAIRS, P, N], bf16, kind="ExternalInput")  # qbias^T + b_q
    kb = nc.dram_tensor("kb", [PAIRS, P, N], bf16, kind="ExternalInput")
    vb = nc.dram_tensor("vb", [N, CL], bf16, kind="ExternalInput")        # vbias + b_v
    wp = nc.dram_tensor("wp", [P, PAIRS, C], bf16, kind="ExternalInput")  # W_proj local rows
    ot = nc.dram_tensor("ot", [C, N], f32, kind="ExternalOutput")        # partial out^T

    xt_r = xt.ap().rearrange("(co p) n -> p co n", p=P)
    wq_r = wq.ap()
    wk_r = wk.ap()
    wv_r = wv.ap()
    wp_r = wp.ap()
    vb_r = vb.ap().rearrange("(mc p) j -> p mc j", p=P)
    ot_r = ot.ap().rearrange("(cc p) n -> p cc n", p=P)

    with tile.TileContext(nc) as tc:
        with (
            tc.tile_pool(name="persist", bufs=1) as pp,
            tc.tile_pool(name="stream", bufs=2) as sp,
            tc.tile_pool(name="ps", bufs=2, space="PSUM") as ps,
        ):
            # ---- persistent tiles ----
            wq_sb = pp.tile([P, KC, CL], bf16)
            wk_sb = pp.tile([P, KC, CL], bf16)
            wv_sb = pp.tile([P, KC, CL], bf16)
            wp_sb = pp.tile([P, PAIRS, C], bf16)
            qT = pp.tile([P, PAIRS, N], f32r)    # q^T (pair-stacked heads)
            kT = pp.tile([P, PAIRS, N], f32r)    # k^T
            v_aug = pp.tile([P, MC, HL, D1], f32r)  # [v | 1] per m-chunk/head
            ones_f32 = pp.tile([P, 1], f32)
            ones_row = pp.tile([1, P], f32)      # stationary for PE broadcast
            pacc = pp.tile([P, C // P, NB], f32)  # nb=3 proj partial acc

            # DMA priority: wq + first xt block feed the first matmuls; wk/wv
            # right behind so the k chains and v chains never wait.
            nc.sync.dma_start(wq_sb[:], wq_r)
            xt_blks = {}

            def fetch_xt(nb):
                t = sp.tile([P, KC, NB], bf16, tag="xt", bufs=2,
                            name=f"xt_{nb}")
                nc.sync.dma_start(t[:], xt_r[:, :, nb * NB:(nb + 1) * NB])
                xt_blks[nb] = t

            fetch_xt(0)

            # PE warmup: ~4us of dense dummy matmuls flips the HAM clock
            # gate to 8/8 (2.4 GHz) while the first DMAs land. (Fewer than
            # ~10 measurably re-throttles the clock: 5 warmups cost +60us.)
            warm_a = pp.tile([P, P], f32r)
            warm_b = pp.tile([P, NB], f32r)
            nc.vector.memset(warm_a.bitcast(f32)[:], 0.0)
            nc.vector.memset(warm_b.bitcast(f32)[:], 0.0)
            wps = ps.tile([P, 2, NB], f32, tag="sps", name="warm_ps")
            for _ in range(11):
                nc.tensor.matmul(wps[:, 0, :], warm_a[:], warm_b[:], start=True, stop=True)

            nc.vector.memset(ones_f32[:], 1.0)
            nc.vector.memset(ones_row[:], 1.0)
            with nc.allow_low_precision(reason="ones column is exact in f32r"):
                nc.vector.tensor_copy(
                    v_aug[:, :, :, HD], ones_f32.to_broadcast([P, MC, HL])
                )

            # ---- attention machinery (emission deferred via emit_steps) ----
            steps = [(nb, c3, mc)
                     for nb in range(NBS)
                     for c3 in range(PAIRS)
                     for mc in range(MC)]
            o_blks = {}
            u_cur = {}
            sps_tiles = {}
            exp_tiles = {}
            s_insts = {}

            def emit_s(i):
                nb, c3, mc = steps[i]
                ns = slice(nb * NB, (nb + 1) * NB)
                ms = slice(mc * P, (mc + 1) * P)
                sps = ps.tile([P, 2, NB], f32, tag="sps", name=f"s_{nb}_{c3}_{mc}")
                sps_tiles[i] = sps
                insts = []
                for hp in range(2):
                    hb = slice(hp * HD, (hp + 1) * HD)
                    bi = nc.tensor.matmul(
                        sps[:, hp, :], kT[hb, c3, ms], qT[hb, c3, ns],
                        start=True, stop=True,
                    )
                    insts.append(bi.ins)
                s_insts[i] = insts

            def emit_exp(i):
                nb, c3, mc = steps[i]
                exps = sp.tile([P, 2, NB], f32r, tag="exps", bufs=6,
                               name=f"e_{nb}_{c3}_{mc}")
                exp_tiles[i] = exps
                nc.scalar.activation(exps[:], sps_tiles.pop(i)[:], EXP, scale=SCALE)

            def emit_av(i):
                nb, c3, mc = steps[i]
                if mc == 0:
                    u_cur[0] = ps.tile([D1, NB], f32, tag="ups", name=f"u_{nb}_{c3}_0")
                    u_cur[1] = ps.tile([D1, NB], f32, tag="ups", name=f"u_{nb}_{c3}_1")
                exps = exp_tiles.pop(i)
                for hp in range(2):
                    bi = nc.tensor.matmul(
                        u_cur[hp][:], v_aug[:, mc, c3 * 2 + hp, :],
                        exps[:, hp, :],
                        start=(mc == 0), stop=(mc == MC - 1),
                    )
                    # Pin PE order: the (independent) S matmuls of step i+2
                    # must precede AV(i) in the PE FIFO. Both gate on exp(i)
                    # (AV on the data, S(i+2) on its sps slot), so S(i+2)'s
                    # stream covers AV's ldweights + semaphore latency.
                    if hp == 0 and i + 2 in s_insts:
                        add_dep_helper(bi.ins, s_insts[i + 2][-1], sync=False,
                                       reason="keep S(i+2) ahead of AV(i)")

            def emit_normalize_final(nb, c3):
                # Tail-latency variant for the very last (nb, c3): the
                # multiply reads U straight from PSUM (fusing the staging
                # copy), so the DVE chain is den,den,rec,rec,mul,mul with
                # the broadcasts on gpsimd in between.
                o_blk = o_blks[nb]
                dens, recs, bcs = [], [], []
                for hp in range(2):
                    den = sp.tile([1, NB], f32, tag="den", bufs=4,
                                  name=f"dF_{hp}")
                    nc.vector.tensor_copy(den[:], u_cur[hp][HD:D1, :])
                    dens.append(den)
                for hp in range(2):
                    rec = sp.tile([1, NB], f32, tag="rec", bufs=4,
                                  name=f"rF_{hp}")
                    nc.vector.reciprocal_approx_fast(rec[:], dens[hp][:])
                    bc = sp.tile([P, NB], f32, tag="bc", bufs=3,
                                 name=f"bcF_{hp}")
                    nc.gpsimd.partition_broadcast(bc[:], rec[:])
                    bcs.append(bc)
                for hp in range(2):
                    hb = slice(hp * HD, (hp + 1) * HD)
                    with nc.allow_low_precision(reason="bf16 proj operand"):
                        nc.vector.tensor_mul(
                            o_blk[hb, c3, :], u_cur[hp][0:HD, :], bcs[hp][hb, :]
                        )

            def emit_normalize(nb, c3):
                if nb == NBS - 1 and c3 == PAIRS - 1:
                    emit_normalize_final(nb, c3)
                    return
                # Drain BOTH U psum banks (den + o_blk copies on DVE, with
                # the reciprocal reading the SBUF-staged den), then run the
                # latency-bound broadcast/mul chain off the critical path.
                # partition_broadcast can only write at base 0, so broadcast
                # to all 128 rows and read the 64-row window matching each
                # head's base (SB-SB ops need equal bases).
                o_blk = o_blks[nb]
                recs, bcs = [], []
                for hp in range(2):
                    u = u_cur[hp]
                    hb = slice(hp * HD, (hp + 1) * HD)
                    # den must be staged to SBUF partition 0 first: the
                    # custom-DVE reciprocal cannot read PSUM at partition
                    # base 64 on hardware (CoreSim accepts it; gpsimd cannot
                    # read PSUM at all).
                    den = sp.tile([1, NB], f32, tag="den", bufs=4,
                                  name=f"d_{nb}_{c3}_{hp}")
                    nc.vector.tensor_copy(den[:], u[HD:D1, :])
                    with nc.allow_low_precision(reason="bf16 proj operand"):
                        nc.vector.tensor_copy(o_blk[hb, c3, :], u[0:HD, :])
                    rec = sp.tile([1, NB], f32, tag="rec", bufs=4,
                                  name=f"r_{nb}_{c3}_{hp}")
                    nc.vector.reciprocal_approx_fast(rec[:], den[:])
                    recs.append(rec)
                for hp in range(2):
                    bc = sp.tile([P, NB], f32, tag="bc", bufs=3,
                                 name=f"bc_{nb}_{c3}_{hp}")
                    nc.gpsimd.partition_broadcast(bc[:], recs[hp][:])
                    bcs.append(bc)
                for hp in range(2):
                    hb = slice(hp * HD, (hp + 1) * HD)
                    with nc.allow_low_precision(reason="bf16 proj operand"):
                        nc.vector.tensor_mul(
                            o_blk[hb, c3, :], o_blk[hb, c3, :], bcs[hp][hb, :]
                        )

            # proj(nb) is cut into 12 pieces (6 matmul-chains + 6 drains),
            # one interleaved into every 4th step of nb+1's stream: a chain's
            # 3 matmuls are emitted back-to-back so they pipeline at-rate in
            # the PE, and the amortized per-step PE cost stays under the exp
            # cadence.
            proj_state = {}

            def proj_pieces(nb):
                ns = slice(nb * NB, (nb + 1) * NB)
                o_blk = o_blks.pop(nb)
                for cc in range(C // P):
                    cs = slice(cc * P, (cc + 1) * P)
                    def chain_piece(cc=cc, cs=cs):
                        mm = ps.tile([P, NB], f32, tag="mps", name=f"p_{nb}_{cc}")
                        proj_state["mm"] = mm
                        for c3 in range(PAIRS):
                            nc.tensor.matmul(
                                mm[:], wp_sb[:, c3, cs], o_blk[:, c3, :],
                                start=(c3 == 0), stop=(c3 == PAIRS - 1),
                            )
                    yield chain_piece
                    def out_piece(cc=cc):
                        mm = proj_state.pop("mm")
                        ot_t = sp.tile([P, NB], f32, tag="ot", bufs=3,
                                       name=f"ot_{nb}_{cc}")
                        nc.vector.tensor_copy(ot_t[:], mm[:])
                        nc.sync.dma_start(ot_r[:, cc, ns], ot_t[:])
                    yield out_piece

            # nb=3 has no following steps to host its proj chains, so its
            # c3=0/c3=1 projection partials run during its own c3=1/c3=2
            # steps, accumulated in SBUF; only the 6 c3=2 matmuls + adds
            # remain after the final normalize.
            def tail_phase_pieces(phase):
                o_blk = o_blks[NBS - 1]
                for cc in range(C // P):
                    cs = slice(cc * P, (cc + 1) * P)
                    def piece(cc=cc, cs=cs, phase=phase):
                        mm = ps.tile([P, NB], f32, tag="mps",
                                     name=f"t_{phase}_{cc}")
                        nc.tensor.matmul(
                            mm[:], wp_sb[:, phase, cs],
                            o_blk[:, phase, :], start=True, stop=True,
                        )
                        if phase == 0:
                            nc.vector.tensor_copy(pacc[:, cc, :], mm[:])
                        else:
                            nc.vector.tensor_add(
                                pacc[:, cc, :], pacc[:, cc, :], mm[:]
                            )
                    yield piece

            state = {"si": 0, "next_s": 0, "pending_proj": None,
                     "tail_parts": None, "step_in_nb": 0}

            def emit_steps(upto):
                for i in range(state["si"], upto):
                    nb, c3, mc = steps[i]
                    if mc == 0 and c3 == 0:
                        o_blks[nb] = sp.tile([P, PAIRS, NB], bf16, tag="oblk",
                                             bufs=2, name=f"o_{nb}")
                        if nb > 0:
                            if state["pending_proj"] is not None:
                                for piece in state["pending_proj"]:
                                    piece()
                            state["pending_proj"] = proj_pieces(nb - 1)
                        state["step_in_nb"] = 0
                    while state["next_s"] <= min(i + 2, upto - 1):
                        emit_s(state["next_s"])
                        state["next_s"] += 1
                    emit_exp(i)
                    emit_av(i)
                    # Normalize first on boundary steps so the U-bank drain
                    # ops aren't queued behind a piece's DVE copy.
                    if mc == MC - 1:
                        emit_normalize(nb, c3)
                    # One proj piece every 4th step (after AV so a piece
                    # stalled on its psum slot delays later work, not AV(i)).
                    if (state["pending_proj"] is not None
                            and state["step_in_nb"] >= 3
                            and state["step_in_nb"] % 4 == 3):
                        piece = next(state["pending_proj"], None)
                        if piece is None:
                            state["pending_proj"] = None
                        else:
                            piece()
                    if nb == NBS - 1 and c3 >= 1 and mc == 3:
                        state["tail_parts"] = tail_phase_pieces(c3 - 1)
                    if (state["tail_parts"] is not None
                            and state["step_in_nb"] % 2 == 1):
                        piece = next(state["tail_parts"], None)
                        if piece is None:
                            state["tail_parts"] = None
                        else:
                            piece()
                    state["step_in_nb"] += 1
                state["si"] = upto

            # ---- prologue: stream x^T in n-blocks; produce q^T, k^T, v.
            # After each block, the newly-unlocked (nb=0, c3=0) attention
            # steps are emitted so their exps run under later blocks' PE
            # work. ----
            for nb in range(NBS):
                ns = slice(nb * NB, (nb + 1) * NB)
                xt_blk = xt_blks.pop(nb)
                if nb > 0 and nb + 1 < NBS:
                    fetch_xt(nb + 1)

                # nb=0 runs all q chains before the k chains so the k matmuls
                # never wait on the wk DMA (which is queued after wq + xt0);
                # the xt1 prefetch is also deferred past the q chains so it
                # doesn't steal DMA bandwidth from wk.
                if nb == 0:
                    qk_order = [(wq_sb, qb, qT, c3) for c3 in range(PAIRS)] + \
                               [(None, None, None, -1)] + \
                               [(wk_sb, kb, kT, c3) for c3 in range(PAIRS)]
                else:
                    qk_order = [(w, bd, d, c3) for c3 in range(PAIRS)
                                for (w, bd, d) in ((wq_sb, qb, qT), (wk_sb, kb, kT))]
                for (w_sb, bias_dram, dst, c3) in qk_order:
                    if w_sb is None:
                        # wk/wv DMAs deferred here so wq + xt0 own the
                        # queues during the warmup window.
                        nc.sync.dma_start(wk_sb[:], wk_r)
                        nc.sync.dma_start(wv_sb[:], wv_r)
                        fetch_xt(1)
                        continue
                    js = slice(c3 * P, (c3 + 1) * P)
                    mm = ps.tile([P, NB], f32, tag="mps", name=f"qk_{nb}_{c3}")
                    for co in range(KC):
                        nc.tensor.matmul(
                            mm[:], w_sb[:, co, js], xt_blk[:, co, :],
                            start=(co == 0), stop=(co == KC - 1),
                        )
                    bias_t = sp.tile([P, NB], bf16, tag="bias", bufs=4,
                                     name=f"b_{nb}_{c3}")
                    nc.sync.dma_start(bias_t[:], bias_dram.ap()[c3, :, ns])
                    with nc.allow_low_precision(reason="f32r matmul operand"):
                        nc.vector.tensor_add(dst[:, c3, ns], mm[:], bias_t[:])

                for ch in range(NB // P):
                    mchunk = nb * (NB // P) + ch
                    cs = slice(ch * P, (ch + 1) * P)
                    mm = ps.tile([P, NB], f32, tag="mps", name=f"v_{nb}_{ch}")
                    for co in range(KC):
                        nc.tensor.matmul(
                            mm[:, :CL], xt_blk[:, co, cs], wv_sb[:, co, :],
                            start=(co == 0), stop=(co == KC - 1),
                        )
                    vb_t = sp.tile([P, CL], bf16, tag="vb", bufs=4,
                                   name=f"vb_{mchunk}")
                    nc.sync.dma_start(vb_t[:], vb_r[:, mchunk, :])
                    with nc.allow_low_precision(reason="f32r matmul operand"):
                        nc.vector.tensor_add(
                            v_aug[:, mchunk, :, 0:HD], mm[:, :CL], vb_t[:]
                        )

                # steps (0, 0, mc) unlock 4 m-chunks per completed xt block
                emit_steps(4 * (nb + 1) if nb < NBS - 1 else 12)
            nc.sync.dma_start(wp_sb[:], wp_r)

            # ---- remaining attention + projection ----
            emit_steps(len(steps))

            # drain remaining pieces, then close the last n-block's
            # projection: 6 single matmuls on the final c3 plus the SBUF
            # accumulator.
            if state["pending_proj"] is not None:
                for piece in state["pending_proj"]:
                    piece()
            if state["tail_parts"] is not None:
                for piece in state["tail_parts"]:
                    piece()
            # The final 6 matmuls drain as 3 pairs through the (now free)
            # sps psum tiles, each closed by one wide add + one wide DMA.
            o_blk3 = o_blks.pop(NBS - 1)
            ns3 = slice((NBS - 1) * NB, NBS * NB)
            for pc in range(C // P // 2):
                mm = ps.tile([P, 2, NB], f32, tag="sps", name=f"tf_{pc}")
                for j in range(2):
                    cc = 2 * pc + j
                    cs = slice(cc * P, (cc + 1) * P)
                    nc.tensor.matmul(mm[:, j, :], wp_sb[:, 2, cs],
                                     o_blk3[:, 2, :], start=True, stop=True)
                ot_t = sp.tile([P, 2, NB], f32, tag="otw", bufs=2,
                               name=f"ot_3_{pc}")
                nc.vector.tensor_add(
                    ot_t[:], pacc[:, 2 * pc:2 * pc + 2, :], mm[:]
                )
                nc.sync.dma_start(ot_r[:, 2 * pc:2 * pc + 2, ns3], ot_t[:])

    nc.compile()
    return nc


def _get_nc():
    if "nc" not in _CACHE:
        _CACHE["nc"] = _build()
    return _CACHE["nc"]


def _prep_in_maps(x, qbias, kbias, vbias, W_qkv, b_qkv, W_proj):
    x = np.asarray(x, dtype=np.float32)
    qbias = np.asarray(qbias, dtype=np.float32)
    kbias = np.asarray(kbias, dtype=np.float32)
    vbias = np.asarray(vbias, dtype=np.float32)
    W_qkv = np.asarray(W_qkv, dtype=np.float32)
    b_qkv = np.asarray(b_qkv, dtype=np.float32)
    W_proj = np.asarray(W_proj, dtype=np.float32)
    b16 = ml_dtypes.bfloat16

    xts = [np.ascontiguousarray(x[b].T).astype(b16) for b in range(B)]
    in_maps = []
    for core in range(8):
        b, hg = core // 2, core % 2
        heads = slice(hg * HL, (hg + 1) * HL)
        qcols = slice(hg * CL, (hg + 1) * CL)
        kcols = slice(C + hg * CL, C + (hg + 1) * CL)
        vcols = slice(2 * C + hg * CL, 2 * C + (hg + 1) * CL)

        # per-head bias + projection bias, transposed to [pair, 128, N]
        qb_ = qbias[b, heads] + b_qkv[qcols].reshape(HL, 1, HD)   # [6, N, 64]
        kb_ = kbias[b, heads] + b_qkv[kcols].reshape(HL, 1, HD)
        qb_t = np.ascontiguousarray(qb_.transpose(0, 2, 1)).reshape(PAIRS, P, N)
        kb_t = np.ascontiguousarray(kb_.transpose(0, 2, 1)).reshape(PAIRS, P, N)
        # v bias in natural [N, 384] (heads side by side, matching Wv columns)
        vb_ = vbias[b, heads] + b_qkv[vcols].reshape(HL, 1, HD)   # [6, N, 64]
        vb_n = np.ascontiguousarray(vb_.transpose(1, 0, 2)).reshape(N, CL)

        def chunked(w, nchunk):  # [nchunk*P, cols] -> [P, nchunk, cols]
            return np.ascontiguousarray(
                w.reshape(nchunk, P, -1).transpose(1, 0, 2)
            ).astype(b16)

        in_maps.append({
            "xt": xts[b],
            "wq": chunked(W_qkv[:, qcols], KC),
            "wk": chunked(W_qkv[:, kcols], KC),
            "wv": chunked(W_qkv[:, vcols], KC),
            "qb": qb_t.astype(b16),
            "kb": kb_t.astype(b16),
            "vb": vb_n.astype(b16),
            "wp": chunked(W_proj[hg * CL:(hg + 1) * CL, :], PAIRS),
        })
    return in_maps


def kernel(x, qbias, kbias, vbias, W_qkv, b_qkv, W_proj, b_proj, **run_kwargs):
    nc = _get_nc()
    in_maps = _prep_in_maps(x, qbias, kbias, vbias, W_qkv, b_qkv, W_proj)
    res = run_bass_kernel_spmd(nc, in_maps, core_ids=list(range(8)), **run_kwargs)
    _CACHE["last_results"] = res

    b_proj = np.asarray(b_proj, dtype=np.float32)
    out = np.empty((B, N, C), dtype=np.float32)
    for b in range(B):
        part = res.results[2 * b]["ot"] + res.results[2 * b + 1]["ot"]  # [C, N]
        out[b] = part.T + b_proj
    return out
